# revision 30
# baseline (speedup 1.0000x reference)
"""ANI-style per-species MLP (MoE hard routing) on 8 TRN2 NeuronCores.

Strategy:
  - Host: flatten atoms, sort by species, pad each species bucket to a
    multiple of 8*128, and deal equal per-species segments to each core.
    Every core runs the SAME graph (SPMD) over its own atoms.
  - Device (per core): tiles of up to 512 atoms run the 4-layer MLP
    (384->160->128->96->1, CELU) with their species' weights only (4x less
    work than the dense reference). Matmuls in bf16 (fp32 PSUM accumulate),
    CELU via one ScalarE Exp pass + one fused custom DVE op:
        celu(v) = relu(v) + (min(exp(v), 1) - 1)       [v = z + b]
    Tiles are processed in quads (shared packed L1-remainder + energy banks)
    and pairs (batched L2/L3 elementwise) through a software-pipelined skew.
  - Host: scatter per-atom energies back, add the L4 bias, per-molecule sum.
"""

import os
from contextlib import ExitStack

import numpy as np
import ml_dtypes

import concourse.bacc as bacc
import concourse.mybir as mybir
import concourse.tile as tile
from concourse.bass_utils import run_bass_kernel_spmd

BF16 = ml_dtypes.bfloat16
F32 = np.float32

N_CORES = 8
TILE = 512
GRAN = 128  # species-segment padding granularity
D_AEV = 384
DH1, DH2, DH3 = 160, 128, 96
N_SPECIES = 4

_ACT = mybir.ActivationFunctionType


# --------------------------------------------------------------------------- #
# Fused CELU custom DVE op: out = (min(in0, 1) - 1) + relu(in1 + s0)
#   in0 = exp(z + b) (SBUF, from ScalarE), in1 = z (PSUM f32), s0 = b.
# Result equals celu(z + b) exactly.
# --------------------------------------------------------------------------- #
def _register_celu_op():
    import concourse.dve_ops as dve_ops
    from concourse.dve_spec import Spec, Src0, Src1, C0, One, relu, minn, lower
    from concourse.dve_uop import DveOpSpec

    name = "CELU1_ANT"
    for op in dve_ops.OPS:
        if op.name == name:
            return op
    spec = Spec(
        body=(minn(Src0, One) - One) + relu(Src1 + C0),
        reference=lambda in0, in1, s0, s1, imm2: (np.minimum(in0, 1.0) - 1.0)
        + np.maximum(in1 + s0, 0.0),
    )
    row = dve_ops._CUSTOM_DVE_ROW_BASE + len(dve_ops.OPS)
    assert row < 0x20, "custom DVE row field overflow"
    shas = {}
    for ver in ("v3", "v4"):
        d = DveOpSpec(name=name, opcode=row, uops=lower(spec, ver=ver), rd1_en=True)
        shas[ver] = d.sha(ver)
    op = dve_ops.DveOp(name, spec, False, shas)
    dve_ops.OPS.append(op)
    dve_ops.CUSTOM_DVE_SPECS[name] = spec
    dve_ops._SUB_OPCODE_FOR_NAME[name] = row
    return op


def _tile_plan(seg_atoms):
    """Per-core tile plan from per-species segment sizes (multiples of GRAN).

    Returns a list of quads; each quad is {"s", "n", "tiles": [tile dicts]}
    with up to 4 equal-size tiles (full 512s), tails get a solo quad.
    """
    quads = []
    a0 = 0
    for s in range(N_SPECIES):
        atoms = int(seg_atoms[s])
        nfull, rem = divmod(atoms, TILE)
        fulls = []
        for _ in range(nfull):
            fulls.append({"s": s, "a0": a0, "n": TILE})
            a0 += TILE
        for base in range(0, nfull, 4):
            quads.append({"s": s, "n": TILE, "tiles": fulls[base : base + 4]})
        if rem:
            quads.append({"s": s, "n": rem, "tiles": [{"s": s, "a0": a0, "n": rem}]})
            a0 += rem
    for q in quads:
        # pairs within the quad: (0,1), (2,3)
        q["pairs"] = [q["tiles"][i : i + 2] for i in range(0, len(q["tiles"]), 2)]
        for j, t in enumerate(q["tiles"]):
            t["j"] = j
            t["quad"] = q
    return quads, a0


# --------------------------------------------------------------------------- #
# Graph builder (one core's SPMD program).
# --------------------------------------------------------------------------- #
def build_graph(seg_atoms):
    celu_op = _register_celu_op()
    dt = mybir.dt
    quads, ncore = _tile_plan(seg_atoms)
    pairs = [p for q in quads for p in q["pairs"]]

    nc = bacc.Bacc("TRN2", target_bir_lowering=False, debug=False)

    x_ext = nc.dram_tensor("x", [128, 3 * ncore], dt.bfloat16, kind="ExternalInput")
    w1_ext = nc.dram_tensor("w1", [128, 12 * DH1], dt.bfloat16, kind="ExternalInput")
    w2a_ext = nc.dram_tensor("w2a", [128, 4 * DH2], dt.bfloat16, kind="ExternalInput")
    w2b_ext = nc.dram_tensor("w2b", [128, 4 * DH2], dt.bfloat16, kind="ExternalInput")
    w3_ext = nc.dram_tensor("w3", [128, 4 * DH3], dt.bfloat16, kind="ExternalInput")
    w4_ext = nc.dram_tensor("w4", [96, 128], dt.bfloat16, kind="ExternalInput")
    b1a_ext = nc.dram_tensor("b1a", [128, 4], dt.float32, kind="ExternalInput")
    b1b_ext = nc.dram_tensor("b1b", [128, 4], dt.float32, kind="ExternalInput")
    b2_ext = nc.dram_tensor("b2", [128, 4], dt.float32, kind="ExternalInput")
    b3_ext = nc.dram_tensor("b3", [96, 4], dt.float32, kind="ExternalInput")
    out_ext = nc.dram_tensor("out", [1, ncore], dt.float32, kind="ExternalOutput")

    with tile.TileContext(nc) as tc, ExitStack() as ctx:
        wpool = ctx.enter_context(tc.tile_pool(name="w", bufs=1))
        xpool = ctx.enter_context(tc.tile_pool(name="x", bufs=10))
        epool = ctx.enter_context(tc.tile_pool(name="e", bufs=3))
        spool = ctx.enter_context(tc.tile_pool(name="s", bufs=3))
        p1a = ctx.enter_context(tc.tile_pool(name="p1a", bufs=2, space="PSUM"))
        p1b = ctx.enter_context(tc.tile_pool(name="p1b", bufs=1, space="PSUM"))
        p2 = ctx.enter_context(tc.tile_pool(name="p2", bufs=1, space="PSUM"))
        p3 = ctx.enter_context(tc.tile_pool(name="p3", bufs=1, space="PSUM"))
        p4 = ctx.enter_context(tc.tile_pool(name="p4", bufs=1, space="PSUM"))

        # --- load weights/biases once. w1 gates the first matmul: split it
        # across partitions AND dispatch from four different engine
        # sequencers (each dma_start costs ~0.6us of sequencer dispatch).
        w1_sb = wpool.tile([128, 12 * DH1], dt.bfloat16)
        for q, eng in enumerate([nc.sync, nc.scalar, nc.gpsimd, nc.sync]):
            eng.dma_start(
                w1_sb[32 * q : 32 * (q + 1), :], w1_ext[32 * q : 32 * (q + 1), :]
            )
        w2a_sb = wpool.tile([128, 4 * DH2], dt.bfloat16)
        w2b_sb = wpool.tile([128, 4 * DH2], dt.bfloat16)
        w3_sb = wpool.tile([128, 4 * DH3], dt.bfloat16)
        w4_sb = wpool.tile([96, 128], dt.bfloat16)
        b1a_sb = wpool.tile([128, 4], dt.float32)
        b1b_sb = wpool.tile([128, 4], dt.float32)
        b2_sb = wpool.tile([128, 4], dt.float32)
        b3_sb = wpool.tile([96, 4], dt.float32)
        for (sb, ext), eng in zip(
            [
                (w2a_sb, w2a_ext), (w2b_sb, w2b_ext), (w3_sb, w3_ext),
                (w4_sb, w4_ext), (b1a_sb, b1a_ext), (b1b_sb, b1b_ext),
                (b2_sb, b2_ext), (b3_sb, b3_ext),
            ],
            [nc.scalar, nc.gpsimd, nc.sync, nc.scalar] * 2,
        ):
            eng.dma_start(sb[:], ext[:])

        def celu(z_ap, bias, shape, tag, P=None):
            P = z_ap.shape[0] if P is None else P
            free = list(z_ap.shape[1:])
            e = epool.tile(shape, dt.bfloat16, tag="e" + tag, name="e" + tag)
            e_ap = e[0:P, 0 : free[0]] if len(free) == 1 else \
                e[0:P, 0 : free[0], 0 : free[1]]
            nc.scalar.activation(e_ap, z_ap, _ACT.Exp, bias=bias)
            sx = spool.tile(shape, dt.bfloat16, tag="s" + tag, name="s" + tag)
            s_ap = sx[0:P, 0 : free[0]] if len(free) == 1 else \
                sx[0:P, 0 : free[0], 0 : free[1]]
            nc.vector._custom_dve(celu_op, out=s_ap, in0=e_ap, in1=z_ap, s0=bias)
            return sx

        def quad_start(q, first=False):
            n = q["n"]
            for t in q["tiles"]:
                xt = xpool.tile([128, 3, TILE], dt.bfloat16, tag="xt", name="xt")
                src = x_ext[:, 3 * t["a0"] : 3 * (t["a0"] + n)].rearrange(
                    "p (c n) -> p c n", c=3
                )
                if first:
                    # fine-grained partition splits across two sequencers so
                    # the first matmul's input lands with minimum latency
                    for h in range(4):
                        eng = nc.gpsimd if h % 2 else nc.sync
                        eng.dma_start(
                            xt[32 * h : 32 * (h + 1), 0:3, 0:n],
                            src[32 * h : 32 * (h + 1), :, :],
                        )
                else:
                    # steady-state x traffic rides the otherwise-idle GpSimd
                    # sequencer, keeping Sync free for output DMAs
                    nc.gpsimd.dma_start(xt[:, 0:3, 0:n], src)
                t["xt"] = xt
            q["z1b"] = p1b.tile([128, TILE], dt.float32, name="z1b", tag="z1b")

        def stage1(pair):
            # L1 main: weight-stationary k-outer over the pair (3 LDWs/pair)
            n = pair[0]["n"]
            s = pair[0]["s"]
            for t in pair:
                t["z1a"] = p1a.tile([128, TILE], dt.float32, name="z1a", tag="z1a")
            for k in range(3):
                base = (s * 3 + k) * DH1
                for t in pair:
                    nc.tensor.matmul(
                        t["z1a"][:, 0:n], w1_sb[:, base : base + 128],
                        t["xt"][:, k, 0:n],
                        start=(k == 0), stop=(k == 2),
                    )
            for t in pair:
                t["s1a"] = celu(
                    t["z1a"][:, 0:n], b1a_sb[:, s : s + 1], [128, TILE], "1a"
                )
            if pair[0]["j"] == 0:
                # L1 remainder for the whole quad, packed into one PSUM bank
                # at partitions 32j
                q = pair[0]["quad"]
                z1b = q["z1b"]
                for j, t in enumerate(q["tiles"]):
                    for k in range(3):
                        base = (s * 3 + k) * DH1 + 128
                        nc.tensor.matmul(
                            z1b[32 * j : 32 * j + 32, 0:n],
                            w1_sb[:, base : base + 32],
                            t["xt"][:, k, 0:n],
                            start=(k == 0), stop=(k == 2),
                            tile_position=(0, 32 * j),
                        )
                P = 32 * len(q["tiles"])
                q["s1b"] = celu(
                    z1b[0:P, 0:n], b1b_sb[0:P, s : s + 1], [128, TILE], "1b"
                )

        def stage2(pair):
            n = pair[0]["n"]
            s = pair[0]["s"]
            np_ = len(pair)
            z2 = p2.tile([128, 2, TILE], dt.float32, name="z2", tag="z2")
            for c, t in enumerate(pair):
                nc.tensor.matmul(
                    z2[:, c, 0:n], w2a_sb[:, s * DH2 : (s + 1) * DH2],
                    t["s1a"][:, 0:n],
                    start=True, stop=False,
                )
                j = t["j"]
                nc.tensor.matmul(
                    z2[:, c, 0:n],
                    w2b_sb[32 * j : 32 * j + 32, s * DH2 : (s + 1) * DH2],
                    t["quad"]["s1b"][32 * j : 32 * j + 32, 0:n],
                    start=False, stop=True,
                    tile_position=(32 * j, 0),
                )
            s2 = celu(
                z2[:, 0:np_, 0:n], b2_sb[:, s : s + 1], [128, 2, TILE], "2"
            )
            for c, t in enumerate(pair):
                t["s2"] = (s2, c)

        def stage3(pair):
            n = pair[0]["n"]
            s = pair[0]["s"]
            np_ = len(pair)
            z3 = p3.tile([96, 2, TILE], dt.float32, name="z3", tag="z3")
            for c, t in enumerate(pair):
                s2, cc = t["s2"]
                nc.tensor.matmul(
                    z3[:, c, 0:n], w3_sb[:, s * DH3 : (s + 1) * DH3],
                    s2[:, cc, 0:n],
                )
            s3 = celu(
                z3[0:96, 0:np_, 0:n], b3_sb[0:96, s : s + 1], [96, 2, TILE], "3"
            )
            for c, t in enumerate(pair):
                t["s3"] = (s3, c)

        def stage4(pair):
            n = pair[0]["n"]
            s = pair[0]["s"]
            q = pair[0]["quad"]
            if pair[0]["j"] == 0:
                q["z4"] = p4.tile([128, TILE], dt.float32, name="z4", tag="z4")
            for t in pair:
                j = t["j"]
                s3, cc = t["s3"]
                # M=32 with zero-padded weight cols: initializes the full
                # stripe, energy in row 32j
                nc.tensor.matmul(
                    q["z4"][32 * j : 32 * j + 32, 0:n],
                    w4_sb[:, s * 32 : (s + 1) * 32],
                    s3[0:96, cc, 0:n],
                    tile_position=(0, 32 * j),
                )
            if pair[-1]["j"] == len(q["tiles"]) - 1:
                gs = len(q["tiles"])
                hi = 32 * (gs - 1) + 1
                en = spool.tile([128, TILE], dt.float32, tag="en", name="en")
                nc.scalar.copy(en[0:hi, 0:n], q["z4"][0:hi, 0:n])
                a0 = q["tiles"][0]["a0"]
                nc.sync.dma_start(
                    out_ext[0:1, a0 : a0 + gs * n].rearrange(
                        "p (a n) -> (p a) n", n=n
                    ),
                    en[0:hi:32, 0:n],
                )

        # pair-granular software pipeline with stage skew
        npairs = len(pairs)
        for step in range(npairs + 3):
            if step < npairs:
                pr = pairs[step]
                if pr[0]["j"] == 0:
                    quad_start(pr[0]["quad"], first=(step == 0))
                stage1(pr)
            if 0 <= step - 1 < npairs:
                stage2(pairs[step - 1])
            if 0 <= step - 2 < npairs:
                stage3(pairs[step - 2])
            if 0 <= step - 3 < npairs:
                stage4(pairs[step - 3])

    nc.compile()
    return nc


# --------------------------------------------------------------------------- #
# Host-side input prep / output unpack.
# --------------------------------------------------------------------------- #
def _prep_weights(W1, b1, W2, b2, W3, b3, W4, b4):
    # w1: [128, 12*DH1], column block (s*3+k) holds W1[s][128k:128k+128, :]
    w1 = np.empty((128, 12 * DH1), BF16)
    for s in range(4):
        for k in range(3):
            base = (s * 3 + k) * DH1
            w1[:, base : base + DH1] = W1[s, 128 * k : 128 * (k + 1), :].astype(BF16)
    w2a = np.empty((128, 4 * DH2), BF16)
    w2b = np.empty((128, 4 * DH2), BF16)  # rem weights replicated at 4 offsets
    w3 = np.empty((128, 4 * DH3), BF16)
    w4 = np.zeros((96, 128), BF16)  # W4[s] in col s*32, zero-padded to M=32
    b1a = np.empty((128, 4), F32)
    b1b = np.empty((128, 4), F32)  # rem bias replicated at 4 offsets
    b2p = np.empty((128, 4), F32)
    b3p = np.empty((96, 4), F32)
    b4p = np.empty(4, F32)
    for s in range(4):
        w2a[:, s * DH2 : (s + 1) * DH2] = W2[s, :128, :].astype(BF16)
        w2b[:, s * DH2 : (s + 1) * DH2] = np.tile(W2[s, 128:, :], (4, 1)).astype(BF16)
        w3[:, s * DH3 : (s + 1) * DH3] = W3[s].astype(BF16)
        w4[:, s * 32] = W4[s, :, 0].astype(BF16)
        b1a[:, s] = b1[s, :128]
        b1b[:, s] = np.tile(b1[s, 128:], 4)
        b2p[:, s] = b2[s]
        b3p[:, s] = b3[s]
        b4p[s] = b4[s, 0]
    return dict(w1=w1, w2a=w2a, w2b=w2b, w3=w3, w4=w4,
                b1a=b1a, b1b=b1b, b2=b2p, b3=b3p), b4p


def _build_x(x_atoms, quads, ncore):
    """Device x layout: tile-contiguous [128, 3*ncore]; within a tile the
    128-partition rows are contiguous runs of (chunk c, atom n)."""
    xh = np.zeros((128, 3 * ncore), BF16)
    for q in quads:
        for t in q["tiles"]:
            a0, n = t["a0"], t["n"]
            blk = x_atoms[a0 : a0 + n].reshape(n, 3, 128).transpose(2, 1, 0)
            xh[:, 3 * a0 : 3 * (a0 + n)] = blk.reshape(128, 3 * n)
    return xh


def _route(species, aev):
    """Sort atoms by species, pad per species to 8*GRAN multiples, deal to
    cores. Returns (x_per_core [8][128,3*ncore] bf16, slotmap [8,ncore],
    seg_atoms [4])."""
    n = species.size
    sp = species.reshape(-1)
    x = aev.reshape(n, D_AEV)
    seg_atoms = []
    per_core_ids = []
    for s in range(N_SPECIES):
        ids = np.nonzero(sp == s)[0]
        seg = max(GRAN, int(np.ceil(len(ids) / (N_CORES * GRAN))) * GRAN)
        seg_atoms.append(seg)
        padded = np.full(N_CORES * seg, -1, np.int64)
        padded[: len(ids)] = ids
        per_core_ids.append(padded.reshape(N_CORES, seg))
    slotmap = np.concatenate(per_core_ids, axis=1)  # [8, ncore]
    ncore = slotmap.shape[1]

    quads, ncore2 = _tile_plan(seg_atoms)
    assert ncore2 == ncore

    x_bf = x.astype(BF16)
    xT = np.empty((N_CORES, 128, 3 * ncore), BF16)
    for i in range(N_CORES):
        xc = np.zeros((ncore, D_AEV), BF16)
        valid = slotmap[i] >= 0
        xc[valid] = x_bf[slotmap[i][valid]]
        xT[i] = _build_x(xc, quads, ncore)
    return xT, slotmap, seg_atoms


_GRAPH_CACHE = {}


def kernel(species, aev, W1, b1, W2, b2, W3, b3, W4, b4):
    species = np.asarray(species)
    aev = np.asarray(aev, F32)
    B, A = species.shape

    xT, slotmap, seg_atoms = _route(species, aev)
    wmap, b4p = _prep_weights(
        np.asarray(W1, F32), np.asarray(b1, F32), np.asarray(W2, F32),
        np.asarray(b2, F32), np.asarray(W3, F32), np.asarray(b3, F32),
        np.asarray(W4, F32), np.asarray(b4, F32),
    )

    key = tuple(seg_atoms)
    if key not in _GRAPH_CACHE:
        _GRAPH_CACHE[key] = build_graph(seg_atoms)
    nc = _GRAPH_CACHE[key]

    in_maps = [{"x": xT[i], **wmap} for i in range(N_CORES)]
    res = run_bass_kernel_spmd(
        nc,
        in_maps,
        core_ids=list(range(N_CORES)),
        trace=bool(os.environ.get("ANI_TRACE")),
    )
    kernel.last_result = res
    if res.exec_time_ns is not None:
        print(f"HW exec time: {res.exec_time_ns} ns")

    n = B * A
    y_atoms = np.zeros(n, F32)
    for i in range(N_CORES):
        valid = slotmap[i] >= 0
        y_atoms[slotmap[i][valid]] = res.results[i]["out"][0][valid]
    y_atoms += b4p[species.reshape(-1)]
    return y_atoms.reshape(B, A).sum(axis=-1).astype(F32)


# revision 33
# speedup vs baseline: 1.1679x; 1.1679x over previous
"""ANI-style per-species MLP (MoE hard routing) on 8 TRN2 NeuronCores.

Strategy:
  - Host: flatten atoms, sort by species, pad each species bucket to a
    multiple of 8*128, and deal equal per-species segments to each core.
    Every core runs the SAME graph (SPMD) over its own atoms.
  - Device (per core): tiles of up to 512 atoms run the 4-layer MLP
    (384->160->128->96->1, CELU) with their species' weights only (4x less
    work than the dense reference). Matmuls in bf16 (fp32 PSUM accumulate),
    CELU via one ScalarE Exp pass + one fused custom DVE op:
        celu(v) = relu(v) + (min(exp(v), 1) - 1)       [v = z + b]
    Tiles are processed in quads (shared packed L1-remainder + energy banks)
    and pairs (batched L2/L3 elementwise) through a software-pipelined skew.
  - Host: scatter per-atom energies back, add the L4 bias, per-molecule sum.
"""

import os
from contextlib import ExitStack

import numpy as np
import ml_dtypes

import concourse.bacc as bacc
import concourse.mybir as mybir
import concourse.tile as tile
from concourse.bass_utils import run_bass_kernel_spmd

BF16 = ml_dtypes.bfloat16
F32 = np.float32

N_CORES = 8
TILE = 512
GRAN = 128  # species-segment padding granularity
D_AEV = 384
DH1, DH2, DH3 = 160, 128, 96
N_SPECIES = 4

_ACT = mybir.ActivationFunctionType


# --------------------------------------------------------------------------- #
# Fused CELU custom DVE op: out = (min(in0, 1) - 1) + relu(in1 + s0)
#   in0 = exp(z + b) (SBUF, from ScalarE), in1 = z (PSUM f32), s0 = b.
# Result equals celu(z + b) exactly.
# --------------------------------------------------------------------------- #
def _register_celu_op():
    import concourse.dve_ops as dve_ops
    from concourse.dve_spec import Spec, Src0, Src1, C0, One, relu, minn, lower
    from concourse.dve_uop import DveOpSpec

    name = "CELU1_ANT"
    for op in dve_ops.OPS:
        if op.name == name:
            return op
    spec = Spec(
        body=(minn(Src0, One) - One) + relu(Src1 + C0),
        reference=lambda in0, in1, s0, s1, imm2: (np.minimum(in0, 1.0) - 1.0)
        + np.maximum(in1 + s0, 0.0),
    )
    row = dve_ops._CUSTOM_DVE_ROW_BASE + len(dve_ops.OPS)
    assert row < 0x20, "custom DVE row field overflow"
    shas = {}
    for ver in ("v3", "v4"):
        d = DveOpSpec(name=name, opcode=row, uops=lower(spec, ver=ver), rd1_en=True)
        shas[ver] = d.sha(ver)
    op = dve_ops.DveOp(name, spec, False, shas)
    dve_ops.OPS.append(op)
    dve_ops.CUSTOM_DVE_SPECS[name] = spec
    dve_ops._SUB_OPCODE_FOR_NAME[name] = row
    return op


def _tile_plan(seg_atoms):
    """Per-core tile plan from per-species segment sizes (multiples of GRAN).

    Returns a list of quads; each quad is {"s", "n", "tiles": [tile dicts]}
    with up to 4 equal-size tiles (full 512s), tails get a solo quad.
    """
    quads = []
    a0 = 0
    for s in range(N_SPECIES):
        atoms = int(seg_atoms[s])
        nfull, rem = divmod(atoms, TILE)
        fulls = []
        for _ in range(nfull):
            fulls.append({"s": s, "a0": a0, "n": TILE})
            a0 += TILE
        for base in range(0, nfull, 4):
            quads.append({"s": s, "n": TILE, "tiles": fulls[base : base + 4]})
        if rem:
            quads.append({"s": s, "n": rem, "tiles": [{"s": s, "a0": a0, "n": rem}]})
            a0 += rem
    for q in quads:
        # pairs within the quad: (0,1), (2,3)
        q["pairs"] = [q["tiles"][i : i + 2] for i in range(0, len(q["tiles"]), 2)]
        for j, t in enumerate(q["tiles"]):
            t["j"] = j
            t["quad"] = q
    return quads, a0


# --------------------------------------------------------------------------- #
# Graph builder (one core's SPMD program).
# --------------------------------------------------------------------------- #
def build_graph(seg_atoms):
    celu_op = _register_celu_op()
    dt = mybir.dt
    quads, ncore = _tile_plan(seg_atoms)
    pairs = [p for q in quads for p in q["pairs"]]

    nc = bacc.Bacc("TRN2", target_bir_lowering=False, debug=False)

    x_ext = nc.dram_tensor("x", [128, 3 * ncore], dt.bfloat16, kind="ExternalInput")
    w1_ext = nc.dram_tensor("w1", [128, 12 * DH1], dt.bfloat16, kind="ExternalInput")
    w2a_ext = nc.dram_tensor("w2a", [128, 4 * DH2], dt.bfloat16, kind="ExternalInput")
    w2b_ext = nc.dram_tensor("w2b", [128, 4 * DH2], dt.bfloat16, kind="ExternalInput")
    w3_ext = nc.dram_tensor("w3", [128, 4 * DH3], dt.bfloat16, kind="ExternalInput")
    w4_ext = nc.dram_tensor("w4", [96, 128], dt.bfloat16, kind="ExternalInput")
    b1a_ext = nc.dram_tensor("b1a", [128, 4], dt.float32, kind="ExternalInput")
    b1b_ext = nc.dram_tensor("b1b", [128, 4], dt.float32, kind="ExternalInput")
    b2_ext = nc.dram_tensor("b2", [128, 4], dt.float32, kind="ExternalInput")
    b3_ext = nc.dram_tensor("b3", [96, 4], dt.float32, kind="ExternalInput")
    out_ext = nc.dram_tensor("out", [1, ncore], dt.float32, kind="ExternalOutput")

    with tile.TileContext(nc) as tc, ExitStack() as ctx:
        wpool = ctx.enter_context(tc.tile_pool(name="w", bufs=1))
        xpool = ctx.enter_context(tc.tile_pool(name="x", bufs=10))
        epool = ctx.enter_context(tc.tile_pool(name="e", bufs=3))
        spool = ctx.enter_context(tc.tile_pool(name="s", bufs=3))
        p1a = ctx.enter_context(tc.tile_pool(name="p1a", bufs=2, space="PSUM"))
        p1b = ctx.enter_context(tc.tile_pool(name="p1b", bufs=1, space="PSUM"))
        p2 = ctx.enter_context(tc.tile_pool(name="p2", bufs=1, space="PSUM"))
        p3 = ctx.enter_context(tc.tile_pool(name="p3", bufs=1, space="PSUM"))
        p4 = ctx.enter_context(tc.tile_pool(name="p4", bufs=1, space="PSUM"))

        # --- load weights/biases once. w1 gates the first matmul: split it
        # across partitions AND dispatch from four different engine
        # sequencers (each dma_start costs ~0.6us of sequencer dispatch).
        w1_sb = wpool.tile([128, 12 * DH1], dt.bfloat16)
        for q, eng in enumerate([nc.sync, nc.scalar, nc.sync, nc.scalar]):
            eng.dma_start(
                w1_sb[32 * q : 32 * (q + 1), :], w1_ext[32 * q : 32 * (q + 1), :]
            )
        w2a_sb = wpool.tile([128, 4 * DH2], dt.bfloat16)
        w2b_sb = wpool.tile([128, 4 * DH2], dt.bfloat16)
        w3_sb = wpool.tile([128, 4 * DH3], dt.bfloat16)
        w4_sb = wpool.tile([96, 128], dt.bfloat16)
        b1a_sb = wpool.tile([128, 4], dt.float32)
        b1b_sb = wpool.tile([128, 4], dt.float32)
        b2_sb = wpool.tile([128, 4], dt.float32)
        b3_sb = wpool.tile([96, 4], dt.float32)
        for (sb, ext), eng in zip(
            [
                (w2a_sb, w2a_ext), (w2b_sb, w2b_ext), (w3_sb, w3_ext),
                (w4_sb, w4_ext), (b1a_sb, b1a_ext), (b1b_sb, b1b_ext),
                (b2_sb, b2_ext), (b3_sb, b3_ext),
            ],
            [nc.scalar, nc.sync, nc.scalar, nc.sync] * 2,
        ):
            eng.dma_start(sb[:], ext[:])

        def celu(z_ap, bias, shape, tag, P=None):
            P = z_ap.shape[0] if P is None else P
            free = list(z_ap.shape[1:])
            e = epool.tile(shape, dt.bfloat16, tag="e" + tag, name="e" + tag)
            e_ap = e[0:P, 0 : free[0]] if len(free) == 1 else \
                e[0:P, 0 : free[0], 0 : free[1]]
            nc.scalar.activation(e_ap, z_ap, _ACT.Exp, bias=bias)
            sx = spool.tile(shape, dt.bfloat16, tag="s" + tag, name="s" + tag)
            s_ap = sx[0:P, 0 : free[0]] if len(free) == 1 else \
                sx[0:P, 0 : free[0], 0 : free[1]]
            nc.vector._custom_dve(celu_op, out=s_ap, in0=e_ap, in1=z_ap, s0=bias)
            return sx

        def quad_start(q, first=False):
            n = q["n"]
            for t in q["tiles"]:
                xt = xpool.tile([128, 3, TILE], dt.bfloat16, tag="xt", name="xt")
                src = x_ext[:, 3 * t["a0"] : 3 * (t["a0"] + n)].rearrange(
                    "p (c n) -> p c n", c=3
                )
                if first:
                    # fine-grained partition splits so the first matmul's
                    # input lands with minimum latency
                    for h in range(4):
                        nc.sync.dma_start(
                            xt[32 * h : 32 * (h + 1), 0:3, 0:n],
                            src[32 * h : 32 * (h + 1), :, :],
                        )
                else:
                    nc.sync.dma_start(xt[:, 0:3, 0:n], src)
                t["xt"] = xt
            q["z1b"] = p1b.tile([128, TILE], dt.float32, name="z1b", tag="z1b")

        def stage1(pair):
            # L1 main: weight-stationary k-outer over the pair (3 LDWs/pair)
            n = pair[0]["n"]
            s = pair[0]["s"]
            for t in pair:
                t["z1a"] = p1a.tile([128, TILE], dt.float32, name="z1a", tag="z1a")
            for k in range(3):
                base = (s * 3 + k) * DH1
                for t in pair:
                    nc.tensor.matmul(
                        t["z1a"][:, 0:n], w1_sb[:, base : base + 128],
                        t["xt"][:, k, 0:n],
                        start=(k == 0), stop=(k == 2),
                    )
            for t in pair:
                t["s1a"] = celu(
                    t["z1a"][:, 0:n], b1a_sb[:, s : s + 1], [128, TILE], "1a"
                )
            if pair[0]["j"] == 0:
                # L1 remainder for the whole quad, packed into one PSUM bank
                # at partitions 32j
                q = pair[0]["quad"]
                z1b = q["z1b"]
                for j, t in enumerate(q["tiles"]):
                    for k in range(3):
                        base = (s * 3 + k) * DH1 + 128
                        nc.tensor.matmul(
                            z1b[32 * j : 32 * j + 32, 0:n],
                            w1_sb[:, base : base + 32],
                            t["xt"][:, k, 0:n],
                            start=(k == 0), stop=(k == 2),
                            tile_position=(0, 32 * j),
                        )
                P = 32 * len(q["tiles"])
                q["s1b"] = celu(
                    z1b[0:P, 0:n], b1b_sb[0:P, s : s + 1], [128, TILE], "1b"
                )

        def stage2(pair):
            n = pair[0]["n"]
            s = pair[0]["s"]
            np_ = len(pair)
            z2 = p2.tile([128, 2, TILE], dt.float32, name="z2", tag="z2")
            for c, t in enumerate(pair):
                nc.tensor.matmul(
                    z2[:, c, 0:n], w2a_sb[:, s * DH2 : (s + 1) * DH2],
                    t["s1a"][:, 0:n],
                    start=True, stop=False,
                )
                j = t["j"]
                nc.tensor.matmul(
                    z2[:, c, 0:n],
                    w2b_sb[32 * j : 32 * j + 32, s * DH2 : (s + 1) * DH2],
                    t["quad"]["s1b"][32 * j : 32 * j + 32, 0:n],
                    start=False, stop=True,
                    tile_position=(32 * j, 0),
                )
            s2 = celu(
                z2[:, 0:np_, 0:n], b2_sb[:, s : s + 1], [128, 2, TILE], "2"
            )
            for c, t in enumerate(pair):
                t["s2"] = (s2, c)

        def stage3(pair):
            n = pair[0]["n"]
            s = pair[0]["s"]
            np_ = len(pair)
            z3 = p3.tile([96, 2, TILE], dt.float32, name="z3", tag="z3")
            for c, t in enumerate(pair):
                s2, cc = t["s2"]
                nc.tensor.matmul(
                    z3[:, c, 0:n], w3_sb[:, s * DH3 : (s + 1) * DH3],
                    s2[:, cc, 0:n],
                )
            s3 = celu(
                z3[0:96, 0:np_, 0:n], b3_sb[0:96, s : s + 1], [96, 2, TILE], "3"
            )
            for c, t in enumerate(pair):
                t["s3"] = (s3, c)

        def stage4(pair):
            n = pair[0]["n"]
            s = pair[0]["s"]
            q = pair[0]["quad"]
            if pair[0]["j"] == 0:
                q["z4"] = p4.tile([128, TILE], dt.float32, name="z4", tag="z4")
            for t in pair:
                j = t["j"]
                s3, cc = t["s3"]
                # M=32 with zero-padded weight cols: initializes the full
                # stripe, energy in row 32j
                nc.tensor.matmul(
                    q["z4"][32 * j : 32 * j + 32, 0:n],
                    w4_sb[:, s * 32 : (s + 1) * 32],
                    s3[0:96, cc, 0:n],
                    tile_position=(0, 32 * j),
                )
            if pair[-1]["j"] == len(q["tiles"]) - 1:
                gs = len(q["tiles"])
                hi = 32 * (gs - 1) + 1
                en = spool.tile([128, TILE], dt.float32, tag="en", name="en")
                nc.scalar.copy(en[0:hi, 0:n], q["z4"][0:hi, 0:n])
                a0 = q["tiles"][0]["a0"]
                nc.sync.dma_start(
                    out_ext[0:1, a0 : a0 + gs * n].rearrange(
                        "p (a n) -> (p a) n", n=n
                    ),
                    en[0:hi:32, 0:n],
                )

        # pair-granular software pipeline with stage skew
        npairs = len(pairs)
        for step in range(npairs + 3):
            if step < npairs:
                pr = pairs[step]
                if pr[0]["j"] == 0:
                    quad_start(pr[0]["quad"], first=(step == 0))
                stage1(pr)
            if 0 <= step - 1 < npairs:
                stage2(pairs[step - 1])
            if 0 <= step - 2 < npairs:
                stage3(pairs[step - 2])
            if 0 <= step - 3 < npairs:
                stage4(pairs[step - 3])

    nc.compile()
    return nc


# --------------------------------------------------------------------------- #
# Host-side input prep / output unpack.
# --------------------------------------------------------------------------- #
def _prep_weights(W1, b1, W2, b2, W3, b3, W4, b4):
    # w1: [128, 12*DH1], column block (s*3+k) holds W1[s][128k:128k+128, :]
    w1 = np.empty((128, 12 * DH1), BF16)
    for s in range(4):
        for k in range(3):
            base = (s * 3 + k) * DH1
            w1[:, base : base + DH1] = W1[s, 128 * k : 128 * (k + 1), :].astype(BF16)
    w2a = np.empty((128, 4 * DH2), BF16)
    w2b = np.empty((128, 4 * DH2), BF16)  # rem weights replicated at 4 offsets
    w3 = np.empty((128, 4 * DH3), BF16)
    w4 = np.zeros((96, 128), BF16)  # W4[s] in col s*32, zero-padded to M=32
    b1a = np.empty((128, 4), F32)
    b1b = np.empty((128, 4), F32)  # rem bias replicated at 4 offsets
    b2p = np.empty((128, 4), F32)
    b3p = np.empty((96, 4), F32)
    b4p = np.empty(4, F32)
    for s in range(4):
        w2a[:, s * DH2 : (s + 1) * DH2] = W2[s, :128, :].astype(BF16)
        w2b[:, s * DH2 : (s + 1) * DH2] = np.tile(W2[s, 128:, :], (4, 1)).astype(BF16)
        w3[:, s * DH3 : (s + 1) * DH3] = W3[s].astype(BF16)
        w4[:, s * 32] = W4[s, :, 0].astype(BF16)
        b1a[:, s] = b1[s, :128]
        b1b[:, s] = np.tile(b1[s, 128:], 4)
        b2p[:, s] = b2[s]
        b3p[:, s] = b3[s]
        b4p[s] = b4[s, 0]
    return dict(w1=w1, w2a=w2a, w2b=w2b, w3=w3, w4=w4,
                b1a=b1a, b1b=b1b, b2=b2p, b3=b3p), b4p


def _build_x(x_atoms, quads, ncore):
    """Device x layout: tile-contiguous [128, 3*ncore]; within a tile the
    128-partition rows are contiguous runs of (chunk c, atom n)."""
    xh = np.zeros((128, 3 * ncore), BF16)
    for q in quads:
        for t in q["tiles"]:
            a0, n = t["a0"], t["n"]
            blk = x_atoms[a0 : a0 + n].reshape(n, 3, 128).transpose(2, 1, 0)
            xh[:, 3 * a0 : 3 * (a0 + n)] = blk.reshape(128, 3 * n)
    return xh


def _route(species, aev):
    """Sort atoms by species, pad per species to 8*GRAN multiples, deal to
    cores. Returns (x_per_core [8][128,3*ncore] bf16, slotmap [8,ncore],
    seg_atoms [4])."""
    n = species.size
    sp = species.reshape(-1)
    x = aev.reshape(n, D_AEV)
    seg_atoms = []
    per_core_ids = []
    for s in range(N_SPECIES):
        ids = np.nonzero(sp == s)[0]
        seg = max(GRAN, int(np.ceil(len(ids) / (N_CORES * GRAN))) * GRAN)
        seg_atoms.append(seg)
        padded = np.full(N_CORES * seg, -1, np.int64)
        padded[: len(ids)] = ids
        per_core_ids.append(padded.reshape(N_CORES, seg))
    slotmap = np.concatenate(per_core_ids, axis=1)  # [8, ncore]
    ncore = slotmap.shape[1]

    quads, ncore2 = _tile_plan(seg_atoms)
    assert ncore2 == ncore

    x_bf = x.astype(BF16)
    xT = np.empty((N_CORES, 128, 3 * ncore), BF16)
    for i in range(N_CORES):
        xc = np.zeros((ncore, D_AEV), BF16)
        valid = slotmap[i] >= 0
        xc[valid] = x_bf[slotmap[i][valid]]
        xT[i] = _build_x(xc, quads, ncore)
    return xT, slotmap, seg_atoms


_GRAPH_CACHE = {}


def kernel(species, aev, W1, b1, W2, b2, W3, b3, W4, b4):
    species = np.asarray(species)
    aev = np.asarray(aev, F32)
    B, A = species.shape

    xT, slotmap, seg_atoms = _route(species, aev)
    wmap, b4p = _prep_weights(
        np.asarray(W1, F32), np.asarray(b1, F32), np.asarray(W2, F32),
        np.asarray(b2, F32), np.asarray(W3, F32), np.asarray(b3, F32),
        np.asarray(W4, F32), np.asarray(b4, F32),
    )

    key = tuple(seg_atoms)
    if key not in _GRAPH_CACHE:
        _GRAPH_CACHE[key] = build_graph(seg_atoms)
    nc = _GRAPH_CACHE[key]

    in_maps = [{"x": xT[i], **wmap} for i in range(N_CORES)]
    res = run_bass_kernel_spmd(
        nc,
        in_maps,
        core_ids=list(range(N_CORES)),
        trace=bool(os.environ.get("ANI_TRACE")),
    )
    kernel.last_result = res
    if res.exec_time_ns is not None:
        print(f"HW exec time: {res.exec_time_ns} ns")

    n = B * A
    y_atoms = np.zeros(n, F32)
    for i in range(N_CORES):
        valid = slotmap[i] >= 0
        y_atoms[slotmap[i][valid]] = res.results[i]["out"][0][valid]
    y_atoms += b4p[species.reshape(-1)]
    return y_atoms.reshape(B, A).sum(axis=-1).astype(F32)
